# revision 1
# baseline (speedup 1.0000x reference)
"""Trainium2 Bass kernel for nn_Attention_71846212928150.

Self-attention block (pre-LN + silu, QKV projections, per-head attention with
q/k LayerNorms, output projection), sharded over 8 NeuronCores by heads:
core c owns heads {2c, 2c+1} = inner columns [128c, 128c+128).

Per core:
  phase 1: stream x in 128-token tiles; LN (bn_stats + quake-rsqrt on DVE,
           no ACT table switches) fused with silu on ACT; PE-transpose;
           fused QKV matmul (fp32r) into [tok, 384] PSUM; evict q/k/v.
  phase 2: partial sum / sumsq of q,k over the local 128 columns;
           AllReduce [128,128] stats across the 8 cores (full 1024-wide LN).
  phase 3: apply q/k LayerNorm in [tok, col] layout (per-partition scalars),
           PE-transpose to [col, tok], apply gain/bias (+ inner**-0.5 folded
           into the q gain on host).
  phase 4: per (batch, head): S^T = K^T.T @ q^T tiles -> exp on ACT (no max
           subtraction: |scores| <~ 1.5 by construction) -> PV matmul with a
           ones-column appended to V so PSUM row 64 accumulates the softmax
           denominator; normalize via reciprocal + PE broadcast.
  phase 5: silu(O) and output projection -> out^T [1024, 4096] partial sums,
           host adds the 8 partials, transposes, adds b_o.
"""

import numpy as np

import concourse.bass as bass
import concourse.mybir as mybir
import concourse.tile as tile
from concourse.masks import make_identity

F32 = mybir.dt.float32
F32R = mybir.dt.float32r
I32 = mybir.dt.int32
AF = mybir.ActivationFunctionType
ALU = mybir.AluOpType
AX = mybir.AxisListType

B = 2
C = 1024
H = 16
DH = 64
INNER = H * DH
NCORES = 8
HL = H // NCORES          # 2 heads per core
CL = HL * DH              # 128 local inner columns
QKV = 3 * CL              # 384
KT = C // 128             # 8 contraction tiles over C
EPS = 1e-5
MAGIC = 0x5F3759DF


def _quake_rsqrt(nc, pool, vpe, shape, suffix=""):
    """rstd = 1/sqrt(vpe) entirely on DVE (fp32-exact after 3 Newton steps)."""
    y = pool.tile(list(shape), F32, name=f"qk_y{suffix}")
    t2 = pool.tile(list(shape), F32, name=f"qk_t2{suffix}")
    nc.vector.tensor_scalar(
        out=y.bitcast(I32), in0=vpe.bitcast(I32), scalar1=1, scalar2=None,
        op0=ALU.logical_shift_right)
    nc.vector.tensor_scalar(
        out=y.bitcast(I32), in0=y.bitcast(I32), scalar1=-1, scalar2=MAGIC,
        op0=ALU.mult, op1=ALU.add)
    for _ in range(3):
        nc.vector.tensor_tensor(out=t2, in0=y, in1=y, op=ALU.mult)
        nc.vector.tensor_tensor(out=t2, in0=t2, in1=vpe, op=ALU.mult)
        nc.vector.tensor_scalar(out=t2, in0=t2, scalar1=-0.5, scalar2=1.5,
                                op0=ALU.mult, op1=ALU.add)
        nc.vector.tensor_tensor(out=y, in0=y, in1=t2, op=ALU.mult)
    return y


def _quake_rsqrt2(nc, pool, vpe, shape, suffix=""):
    """Two-iteration variant (~4e-6 rel err) for the latency-critical x path."""
    y = pool.tile(list(shape), F32, name=f"qj_y{suffix}")
    t2 = pool.tile(list(shape), F32, name=f"qj_t2{suffix}")
    nc.vector.tensor_scalar(
        out=y.bitcast(I32), in0=vpe.bitcast(I32), scalar1=1, scalar2=None,
        op0=ALU.logical_shift_right)
    nc.vector.tensor_scalar(
        out=y.bitcast(I32), in0=y.bitcast(I32), scalar1=-1, scalar2=MAGIC,
        op0=ALU.mult, op1=ALU.add)
    for _ in range(2):
        nc.vector.tensor_tensor(out=t2, in0=y, in1=y, op=ALU.mult)
        nc.vector.tensor_tensor(out=t2, in0=t2, in1=vpe, op=ALU.mult)
        nc.vector.tensor_scalar(out=t2, in0=t2, scalar1=-0.5, scalar2=1.5,
                                op0=ALU.mult, op1=ALU.add)
        nc.vector.tensor_tensor(out=y, in0=y, in1=t2, op=ALU.mult)
    return y


def _fixup_module(nc):
    """Adapt Tile-emitted BIR to this container's walrus build.

    1. The tail `EVENT_SEMAPHORE_RANGE_CLEAR` InstISA (opcode 176) is not
       understood by this walrus' birverifier. Replace it with one
       EventSemaphore sem-write-0 per semaphore in the cleared range
       (functionally equivalent, re-execution stays safe).
    2. Drain instructions carrying more than one semaphore wait fail codegen
       ("Too many sync wait commands"). Hoist the extra waits into standalone
       EventSemaphore wait instructions just before the drain.
    """
    for f in nc.m.functions:
        for bb in f.blocks:
            newlist = []
            changed = False
            for ins in bb.instructions:
                tn = type(ins).__name__
                if tn == "InstISA" and getattr(ins, "isa_opcode", None) == 176:
                    ad = ins.ant_dict or {}
                    first = ad.get("range_first")
                    last = ad.get("range_last")
                    if first is not None and last is not None:
                        si = ins.sync_info
                        sems = list(range(first, last + 1))
                        for k, sem in enumerate(sems):
                            ev = mybir.InstEventSemaphore(
                                name=f"{ins.name}-clr{k}", engine=ins.engine,
                                ins=[], outs=[])
                            upd = mybir.SyncUpdate(
                                sync_type="semaphore", id=sem,
                                update_mode="sem-wr-imm", update_value=0)
                            on_wait = (list(si.on_wait)
                                       if (k == 0 and si is not None and si.on_wait)
                                       else [])
                            ev.sync_info = mybir.SyncInfo(
                                on_wait=on_wait, on_update=[upd])
                            newlist.append(ev)
                        if si is not None and si.on_update:
                            evf = mybir.InstEventSemaphore(
                                name=f"{ins.name}-clrf", engine=ins.engine,
                                ins=[], outs=[])
                            evf.sync_info = mybir.SyncInfo(
                                on_wait=[], on_update=list(si.on_update))
                            newlist.append(evf)
                    changed = True
                    continue
                si = ins.sync_info
                if (si is not None and si.on_wait is not None
                        and len(si.on_wait) > 1):
                    waits = list(si.on_wait)
                    for i, w in enumerate(waits[1:]):
                        ev = mybir.InstEventSemaphore(
                            name=f"{ins.name}-hw{i}", engine=ins.engine,
                            ins=[], outs=[])
                        ev.sync_info = mybir.SyncInfo(on_wait=[w], on_update=[])
                        newlist.append(ev)
                    si.on_wait = [waits[0]]
                    ins.sync_info = si
                    changed = True
                newlist.append(ins)
            if changed:
                bb.instructions = newlist
    return nc


def build_bass(n_tok_per_batch, n_cores=NCORES):
    N = n_tok_per_batch
    T = B * N
    NT = T // 128             # token tiles
    KB = N // 128             # key tiles per batch
    QC = max(1, N // 512)     # q chunks per batch
    QCW = min(512, N)         # q chunk width
    OTC = max(1, T // 512)    # out-proj token chunks
    OTW = min(512, T)

    nc = bass.Bass(trn_type="TRN2", num_devices=n_cores)

    x = nc.dram_tensor("x", [T, C], F32, kind="ExternalInput")
    w_all = nc.dram_tensor("w_all", [C, QKV], F32R, kind="ExternalInput")
    b_all = nc.dram_tensor("b_all", [1, QKV], F32, kind="ExternalInput")
    gbe = nc.dram_tensor("gbe", [128, 4], F32, kind="ExternalInput")
    w_o_loc = nc.dram_tensor("w_o_loc", [CL, C], F32R, kind="ExternalInput")
    out_t = nc.dram_tensor("out_t", [C, T], F32, kind="ExternalOutput")

    with tile.TileContext(nc) as tc:
        _body(tc, x, w_all, b_all, gbe, w_o_loc, out_t,
              N=N, T=T, NT=NT, KB=KB, QC=QC, QCW=QCW, OTC=OTC, OTW=OTW,
              n_cores=n_cores)
    return _fixup_module(nc)


def _body(tc, x, w_all, b_all, gbe, w_o_loc, out_t,
          N, T, NT, KB, QC, QCW, OTC, OTW, n_cores):
    nc = tc.nc

    from contextlib import ExitStack
    octx = ExitStack()
    persist = octx.enter_context(tc.tile_pool(name="persist", bufs=1))

    ident = persist.tile([128, 128], F32)
    make_identity(nc, ident)

    w_all_sb = persist.tile([128, KT, QKV], F32R)
    for kt in range(KT):
        nc.sync.dma_start(out=w_all_sb[:, kt, :],
                          in_=w_all[kt * 128:(kt + 1) * 128, :])
    b_sb = persist.tile([128, QKV], F32)
    nc.sync.dma_start(out=b_sb, in_=b_all.ap().to_broadcast([128, QKV]))
    gbe_sb = persist.tile([128, 4], F32)
    nc.sync.dma_start(out=gbe_sb, in_=gbe[:, :])
    w_o_sb = persist.tile([128, C], F32R)
    nc.sync.dma_start(out=w_o_sb, in_=w_o_loc[:, :])

    qT = persist.tile([128, T], F32R)      # [local col, token]
    kTt = persist.tile([128, T], F32R)
    v_aug = persist.tile([128, NT, 130], F32R)   # [tok%128, tile, head-block]
    q_pre = persist.tile([128, NT, 128], F32)   # [tok%128, tile, local col]
    k_pre = persist.tile([128, NT, 128], F32)
    stats = persist.tile([128, 4 * NT], F32)
    stats_all = persist.tile([128, 4 * NT], F32)

    ones_col = persist.tile([128, NT], F32)
    nc.vector.memset(ones_col, 1.0)
    nc.vector.tensor_copy(out=v_aug[:, :, 64:65], in_=ones_col)
    nc.vector.tensor_copy(out=v_aug[:, :, 129:130], in_=ones_col)

    # ---------------- phase 1: x-side LN+silu, transpose, QKV ----------------
    GB = 4  # token tiles per group
    with tc.tile_pool(name="ph1", bufs=3) as ph1, \
         tc.tile_pool(name="ph1s", bufs=4) as ph1s, \
         tc.tile_pool(name="ph1p", bufs=2, space="PSUM") as ph1p, \
         tc.tile_pool(name="ph1q", bufs=3, space="PSUM") as ph1q:
        for g in range(NT // GB):
            xg = ph1.tile([128, GB, C], F32, name="xg")
            nc.sync.dma_start(
                out=xg,
                in_=x[g * GB * 128:(g + 1) * GB * 128, :].rearrange(
                    "(t p) c -> p t c", p=128))

            stats6 = ph1s.tile([128, GB, 2, 6], F32, name="stats6")
            for t in range(GB):
                for h2 in range(2):
                    nc.vector.bn_stats(out=stats6[:, t, h2, :],
                                       in_=xg[:, t, h2 * 512:(h2 + 1) * 512])
            mv = ph1s.tile([128, GB, 2], F32, name="mv")
            for t in range(GB):
                nc.vector.bn_aggr(out=mv[:, t, :], in_=stats6[:, t, :, :])

            vpe = ph1s.tile([128, GB, 1], F32, name="vpe")
            nc.vector.tensor_scalar(out=vpe, in0=mv[:, :, 1:2], scalar1=EPS,
                                    scalar2=None, op0=ALU.add)
            rstd = _quake_rsqrt2(nc, ph1s, vpe, (128, GB, 1))
            nmr = ph1s.tile([128, GB, 1], F32, name="nmr")
            nc.vector.tensor_tensor(out=nmr, in0=mv[:, :, 0:1], in1=rstd,
                                    op=ALU.mult)
            nc.vector.tensor_scalar(out=nmr, in0=nmr, scalar1=-1.0,
                                    scalar2=None, op0=ALU.mult)

            # silu(LN(x)) in place to keep SBUF within budget
            for t in range(GB):
                nc.scalar.activation(out=xg[:, t, :], in_=xg[:, t, :],
                                     func=AF.Silu,
                                     bias=nmr[:, t, :],
                                     scale=rstd[:, t, :])

            for t in range(GB):
                tt = g * GB + t
                pxT = ph1p.tile([128, 1024], F32, name="pxT")
                for j in range(KT):
                    nc.tensor.transpose(pxT[:, j * 128:(j + 1) * 128],
                                        xg[:, t, j * 128:(j + 1) * 128],
                                        ident)
                xsT = ph1.tile([128, 1024], F32R, name="xsT")
                if t % 2 == 0:
                    nc.vector.tensor_copy(out=xsT, in_=pxT)
                else:
                    nc.scalar.copy(out=xsT, in_=pxT)

                pqkv = ph1q.tile([128, 512], F32, name="pqkv")
                for kt in range(KT):
                    nc.tensor.matmul(
                        pqkv[:, 0:QKV],
                        lhsT=xsT[:, kt * 128:(kt + 1) * 128],
                        rhs=w_all_sb[:, kt, :],
                        start=(kt == 0), stop=(kt == KT - 1))

                nc.vector.scalar_tensor_tensor(
                    out=q_pre[:, tt, :], in0=pqkv[:, 0:128], scalar=1.0,
                    in1=b_sb[:, 0:128], op0=ALU.mult, op1=ALU.add)
                nc.vector.scalar_tensor_tensor(
                    out=k_pre[:, tt, :], in0=pqkv[:, 128:256], scalar=1.0,
                    in1=b_sb[:, 128:256], op0=ALU.mult, op1=ALU.add)
                nc.vector.scalar_tensor_tensor(
                    out=v_aug[:, tt, :].rearrange("p (h e) -> p h e", e=65)[:, :, 0:64],
                    in0=pqkv[:, 256:384].rearrange("p (h e) -> p h e", e=64),
                    scalar=1.0,
                    in1=b_sb[:, 256:384].rearrange("p (h e) -> p h e", e=64),
                    op0=ALU.mult, op1=ALU.add)

    # ---------------- phase 2: q/k stats + AllReduce ----------------
    with tc.tile_pool(name="ph2", bufs=1) as ph2:
        nc.vector.tensor_reduce(out=stats[:, 0:NT], in_=q_pre[:, :, :],
                                axis=AX.X, op=ALU.add)
        nc.vector.tensor_reduce(out=stats[:, 2 * NT:3 * NT], in_=k_pre[:, :, :],
                                axis=AX.X, op=ALU.add)
        scr = ph2.tile([128, 128], F32, name="scr")
        for tt in range(NT):
            nc.scalar.activation(
                out=scr, in_=q_pre[:, tt, :], func=AF.Square,
                accum_out=stats[:, NT + tt:NT + tt + 1])
        for tt in range(NT):
            nc.scalar.activation(
                out=scr, in_=k_pre[:, tt, :], func=AF.Square,
                accum_out=stats[:, 3 * NT + tt:3 * NT + tt + 1])

        with tc.tile_pool(name="dram", bufs=1, space="DRAM") as dpool:
            cc_in = dpool.tile([128, 4 * NT], F32, name="cc_in")
            cc_out = dpool.tile([128, 4 * NT], F32, name="cc_out",
                                addr_space="Shared")
            nc.sync.dma_start(out=cc_in, in_=stats)
            nc.gpsimd.collective_compute(
                "AllReduce", ALU.add,
                replica_groups=[list(range(n_cores))],
                ins=[cc_in.opt()], outs=[cc_out.opt()])
            nc.sync.dma_start(out=stats_all, in_=cc_out)

        # per-token mean / rstd for q and k (over full 1024-wide inner dim)
        qk_stats = []
        for which in range(2):  # 0 -> q, 1 -> k
            s_sum = stats_all[:, 2 * which * NT:(2 * which + 1) * NT]
            s_ssq = stats_all[:, (2 * which + 1) * NT:(2 * which + 2) * NT]
            m = ph2.tile([128, NT], F32, name=f"m_{which}")
            nc.vector.tensor_scalar(out=m, in0=s_sum, scalar1=1.0 / INNER,
                                    scalar2=None, op0=ALU.mult)
            msq = ph2.tile([128, NT], F32, name=f"msq_{which}")
            nc.vector.tensor_scalar(out=msq, in0=s_ssq, scalar1=1.0 / INNER,
                                    scalar2=None, op0=ALU.mult)
            tmp = ph2.tile([128, NT], F32, name=f"tmp_{which}")
            nc.vector.tensor_tensor(out=tmp, in0=m, in1=m, op=ALU.mult)
            nc.vector.tensor_tensor(out=tmp, in0=msq, in1=tmp, op=ALU.subtract)
            nc.vector.tensor_scalar(out=tmp, in0=tmp, scalar1=EPS,
                                    scalar2=None, op0=ALU.add)
            rstd = _quake_rsqrt(nc, ph2, tmp, (128, NT), suffix=f"_{which}")
            nmr = ph2.tile([128, NT], F32, name=f"nmr_{which}")
            nc.vector.tensor_tensor(out=nmr, in0=m, in1=rstd, op=ALU.mult)
            nc.vector.tensor_scalar(out=nmr, in0=nmr, scalar1=-1.0,
                                    scalar2=None, op0=ALU.mult)
            qk_stats.append((m, rstd, nmr))

        # ---------------- phase 3: apply LN, transpose q/k ----------------
        with tc.tile_pool(name="ph3", bufs=8) as ph3, \
             tc.tile_pool(name="ph3p", bufs=4, space="PSUM") as ph3p:
            for which, (pre, dst, gcol) in enumerate(
                    [(q_pre, qT, 0), (k_pre, kTt, 2)]):
                m, rstd, nmr = qk_stats[which]
                for tt in range(NT):
                    qn = ph3.tile([128, 128], F32, name="qn")
                    if which == 1:
                        # k: normalize on ACT so DVE and ACT each carry one
                        # of the two per-tile passes
                        nc.scalar.activation(
                            out=qn, in_=pre[:, tt, :], func=AF.Identity,
                            bias=nmr[:, tt:tt + 1],
                            scale=rstd[:, tt:tt + 1])
                    else:
                        nc.vector.tensor_scalar(
                            out=qn, in0=pre[:, tt, :],
                            scalar1=m[:, tt:tt + 1],
                            scalar2=rstd[:, tt:tt + 1],
                            op0=ALU.subtract, op1=ALU.mult)
                    pq = ph3p.tile([128, 128], F32, name="pq")
                    nc.tensor.transpose(pq, qn, ident)
                    if which == 0:
                        nc.scalar.activation(
                            out=dst[:, tt * 128:(tt + 1) * 128], in_=pq,
                            func=AF.Identity,
                            bias=gbe_sb[:, gcol + 1:gcol + 2],
                            scale=gbe_sb[:, gcol:gcol + 1])
                    else:
                        nc.vector.tensor_scalar(
                            out=dst[:, tt * 128:(tt + 1) * 128], in0=pq,
                            scalar1=gbe_sb[:, gcol:gcol + 1],
                            scalar2=gbe_sb[:, gcol + 1:gcol + 2],
                            op0=ALU.mult, op1=ALU.add)

    # ---------------- phase 4: attention ----------------
    att45 = octx.enter_context(tc.tile_pool(name="att45", bufs=1))
    onorm = att45.tile([128, T], F32)
    siluo = att45.tile([128, T], F32R)
    with tc.tile_pool(name="att", bufs=3) as att, \
         tc.tile_pool(name="dramsc", bufs=2, space="DRAM") as dramsc, \
         tc.tile_pool(name="attp", bufs=2, space="PSUM") as attp, \
         tc.tile_pool(name="attpo", bufs=1, space="PSUM") as attpo:
        NHALF = min(2, QC)            # chunk groups per key tile
        HC = QC // NHALF              # q chunks per group
        for b in range(B):
            for h in range(HL):
                pO = attpo.tile([128, QC, QCW], F32, name="pO", tag="pO")
                for kb in range(KB):
                    vt = b * KB + kb
                    for g in range(NHALF):
                        # two [128, HC*QCW] score tiles double-buffer so the
                        # S matmuls of the next group overlap this group's exp
                        pS = attp.tile([128, HC * QCW], F32, name="pS",
                                       tag="pS")
                        for qi in range(HC):
                            qc = g * HC + qi
                            nc.tensor.matmul(
                                pS[:, qi * QCW:(qi + 1) * QCW],
                                lhsT=kTt[h * 64:(h + 1) * 64,
                                         b * N + kb * 128:
                                         b * N + (kb + 1) * 128],
                                rhs=qT[h * 64:(h + 1) * 64,
                                       b * N + qc * QCW:
                                       b * N + (qc + 1) * QCW],
                                start=True, stop=True)
                        eS = att.tile([128, HC * QCW], F32R, name="eS")
                        nc.scalar.activation(out=eS, in_=pS, func=AF.Exp)
                        for qi in range(HC):
                            qc = g * HC + qi
                            nc.tensor.matmul(
                                pO[0:65, qc, :],
                                lhsT=v_aug[:, vt, h * 65:(h + 1) * 65],
                                rhs=eS[:, qi * QCW:(qi + 1) * QCW],
                                start=(kb == 0), stop=(kb == KB - 1))

                dn = att.tile([1, QC, QCW], F32, name="dn")
                nc.vector.reciprocal(out=dn, in_=pO[64:65, :, :])
                dn_dram = dramsc.tile([1, QC, QCW], F32, name="dn_dram")
                nc.sync.dma_start(out=dn_dram, in_=dn)
                dnb = att.tile([64, QC, QCW], F32, name="dnb")
                nc.sync.dma_start(out=dnb, in_=dn_dram.to_broadcast([64, QC, QCW]))
                nc.vector.tensor_tensor(
                    out=onorm[h * 64:(h + 1) * 64, b * N:(b + 1) * N],
                    in0=pO[0:64, :, :], in1=dnb, op=ALU.mult)

    # ---------------- phase 5: silu(O) + output projection ----------------
    with tc.tile_pool(name="ph5", bufs=4) as ph5, \
         tc.tile_pool(name="ph5p", bufs=4, space="PSUM") as ph5p:
        for half in range(max(1, T // 2048)):
            w = min(2048, T)
            nc.scalar.activation(out=siluo[:, half * w:(half + 1) * w],
                                 in_=onorm[:, half * w:(half + 1) * w],
                                 func=AF.Silu)
        for ct in range(KT):
            for tk in range(OTC):
                po = ph5p.tile([128, OTW], F32, name="po")
                nc.tensor.matmul(
                    po,
                    lhsT=w_o_sb[:, ct * 128:(ct + 1) * 128],
                    rhs=siluo[:, tk * OTW:(tk + 1) * OTW],
                    start=True, stop=True)
                ev = ph5.tile([128, OTW], F32, name="ev")
                if (ct * OTC + tk) % 2 == 0:
                    nc.vector.tensor_copy(out=ev, in_=po)
                else:
                    nc.scalar.copy(out=ev, in_=po)
                nc.sync.dma_start(
                    out=out_t[ct * 128:(ct + 1) * 128,
                              tk * OTW:(tk + 1) * OTW],
                    in_=ev)

    octx.close()


def make_in_maps(inputs, n_tok_per_batch, n_cores=NCORES):
    """Slice full inputs into per-core input maps (head sharding)."""
    x = np.ascontiguousarray(np.asarray(inputs["x"], np.float32)
                             .reshape(B * n_tok_per_batch, C))
    w_q = np.asarray(inputs["w_q"], np.float32)
    w_k = np.asarray(inputs["w_k"], np.float32)
    w_v = np.asarray(inputs["w_v"], np.float32)
    b_q = np.asarray(inputs["b_q"], np.float32)
    b_k = np.asarray(inputs["b_k"], np.float32)
    b_v = np.asarray(inputs["b_v"], np.float32)
    g_q = np.asarray(inputs["g_q"], np.float32)
    be_q = np.asarray(inputs["be_q"], np.float32)
    g_k = np.asarray(inputs["g_k"], np.float32)
    be_k = np.asarray(inputs["be_k"], np.float32)
    w_o = np.asarray(inputs["w_o"], np.float32)

    scale = float(INNER) ** -0.5
    in_maps = []
    for c in range(n_cores):
        cols = slice(c * CL, (c + 1) * CL)
        w_all = np.ascontiguousarray(
            np.concatenate([w_q[:, cols], w_k[:, cols], w_v[:, cols]], axis=1))
        b_all = np.ascontiguousarray(
            np.concatenate([b_q[cols], b_k[cols], b_v[cols]])[None, :])
        gbe = np.ascontiguousarray(
            np.stack([g_q[cols] * scale, be_q[cols] * scale,
                      g_k[cols], be_k[cols]], axis=1))
        w_o_c = np.ascontiguousarray(w_o[cols, :])
        in_maps.append({
            "x": x, "w_all": w_all, "b_all": b_all,
            "gbe": gbe, "w_o_loc": w_o_c,
        })
    return in_maps


def combine_outputs(out_ts, inputs, n_tok_per_batch):
    b_o = np.asarray(inputs["b_o"], np.float32)
    acc = np.zeros_like(out_ts[0], dtype=np.float64)
    for o in out_ts:
        acc += o.astype(np.float64)
    out = acc.T.astype(np.float32) + b_o[None, :]
    return out.reshape(B, n_tok_per_batch, C).astype(np.float32)


_NC_CACHE = {}


def kernel(**inputs):
    from concourse.bass_utils import run_bass_kernel_spmd

    n_tok = np.asarray(inputs["x"]).shape[1]
    if n_tok not in _NC_CACHE:
        _NC_CACHE[n_tok] = build_bass(n_tok)
    nc = _NC_CACHE[n_tok]
    in_maps = make_in_maps(inputs, n_tok)
    res = run_bass_kernel_spmd(nc, in_maps, core_ids=list(range(NCORES)))
    out_ts = [r["out_t"] for r in res.results]
    return combine_outputs(out_ts, inputs, n_tok)

